# revision 13
# baseline (speedup 1.0000x reference)
"""Trainium2 Bass kernel for CompositionalFC (moe_routing).

Reference computation:
    z[n,b,o] = x[b,i] @ weight[n,i,o] + bias[n,o]
    out[b,o] = relu( sum_n comp_weight[b,n] * z[n,b,o] )

Strategy: data-parallel over batch across 8 NeuronCores (512 rows each,
weight/bias replicated), with the expert matmuls in fp8e4 DoubleRow mode
(2 contraction rows per PE pass = 2x bf16 matmul throughput, and half the
weight DMA traffic). Steady state measured at 216 ns per 512-col DoubleRow
matmul == the fp8 PE roofline (~157 TF/s effective).

Accuracy: fp8e4 has a 3-bit mantissa, too coarse for w ~ U[0,1) directly
(~3.4% rel err vs the 2e-2 gate). Mean-centering fixes it: w = 0.5 + v
with v ~ U[-.5,.5); quantize v to fp8 and add the exact rank-1 term
    0.5 * rowsum(x)[b] * (sum_n c[b,n]),
which also dominates the output magnitude. x ships as fp8 (matmul
operand) plus a bf16 batch-major copy from which rowsum(x) is computed
by a single DVE reduce, keeping the rank-1 path off the PE queue.
Measured end-to-end l2 rel err: 7.6e-3.

Per core: z_n accumulates in PSUM over 4 DoubleRow K-tiles of 256, then
one fused combine op per expert: acc = z*c[:,n] + acc. The bias term
(comp_weight @ bias) seeds the accumulators via K=16 bf16 matmuls (hidden
under the startup DMA window); the rank-1 term is added after pair 1,
off both the startup and drain critical paths. ReLU on the way out.

Engine placement: combines (PSUM readers) live on the Vector engine;
the Scalar engine seeds the accumulators from PSUM and fuses the rank-1
term into the final ReLU (bias AP), keeping the drain chain short.
GpSimd cannot access PSUM on TRN2. Each stationary xh tile serves
2 experts x 2 PSUM banks (4 matmuls per LdWeights); the 8 PSUM banks
split 4/4 between two (pair, bt) groups so combine drain overlaps PE
streaming. Weight pair DMAs interleave the two experts' K-chunks (the
kt loop needs both experts' chunk kt first), and pairs 2+ prefetch two
pairs ahead through a 4-deep pool.
"""

import sys

for _p in ("/opt/trn_rl_repo",):
    if _p not in sys.path:
        sys.path.insert(0, _p)

from contextlib import ExitStack

import ml_dtypes
import numpy as np

import concourse.bass as bass
import concourse.mybir as mybir
import concourse.tile as tile
from concourse import bacc
from concourse.bass_utils import run_bass_kernel_spmd

N_CORES = 8
BATCH, IN_DIM, OUT_DIM, N_EXP = 4096, 1024, 1024, 16
BS = BATCH // N_CORES          # 512 batch rows per core
P = 128                        # partitions
BT = BS // P                   # 4 batch tiles per core
KT2 = IN_DIM // 256            # 4 DoubleRow contraction tiles (K=256 each)
FD = 512                       # matmul free dim / PSUM bank width (fp32)
NO = OUT_DIM // FD             # 2 output column tiles
NPAIR = N_EXP // 2             # expert pairs sharing a stationary tile

F32 = mybir.dt.float32
BF16 = mybir.dt.bfloat16
F8 = mybir.dt.float8e4
DR = mybir.MatmulPerfMode.DoubleRow

E4NP = ml_dtypes.float8_e4m3   # TRN fp8e4 == IEEE e4m3 (max 240)


def _build_kernel():
    nc = bacc.Bacc(
        "TRN2",
        target_bir_lowering=False,
        debug=False,
        num_devices=N_CORES,
    )
    # k = kt2*256 + slot*128 + p; b = bt*128 + p_out
    xh8 = nc.declare_dram_parameter("xh8", [P, KT2, 2, BS], F8, isOutput=False)
    xb16 = nc.declare_dram_parameter("xb16", [P, BT, IN_DIM], BF16, isOutput=False)
    w8 = nc.declare_dram_parameter("w8", [N_EXP, P, KT2, 2, OUT_DIM], F8, isOutput=False)
    c = nc.declare_dram_parameter("c", [P, BT, N_EXP], F32, isOutput=False)
    cT = nc.declare_dram_parameter("cT", [N_EXP, BS], BF16, isOutput=False)
    bias = nc.declare_dram_parameter("bias", [N_EXP, OUT_DIM], BF16, isOutput=False)
    out = nc.declare_dram_parameter("out", [P, BT, OUT_DIM], F32, isOutput=True)

    with ExitStack() as ctx:
        tc = ctx.enter_context(tile.TileContext(nc))
        const = ctx.enter_context(tc.tile_pool(name="const", bufs=1))
        accp = ctx.enter_context(tc.tile_pool(name="accp", bufs=1))
        wpool = ctx.enter_context(tc.tile_pool(name="wpool", bufs=6))
        psum = ctx.enter_context(tc.tile_pool(name="psum", bufs=2, space="PSUM"))

        # --- persistent SBUF state -------------------------------------
        # DMA issue order IS the startup critical path (each dma_start
        # costs ~650ns of sync-queue issue time): cT+bias gate the seeds,
        # xh + the first interleaved w chunks gate the main loop; c/xl are
        # needed much later (rowsum runs after pair 0).
        # the GpSimd DMA queue has several-us completion latency, so all
        # startup DMAs stay on sync, smallest/most-gating first
        cT_sb = const.tile([N_EXP, BS], BF16, tag="cT_sb")
        nc.sync.dma_start(cT_sb[:], cT[:, :])
        bias_sb = const.tile([N_EXP, OUT_DIM], BF16, tag="bias_sb")
        nc.sync.dma_start(bias_sb[:], bias[:, :])
        c_sb = const.tile([P, BT, N_EXP], F32, tag="c_sb")
        nc.sync.dma_start(c_sb[:], c[:, :])
        xh_sb = const.tile([P, KT2, 2, BS], F8, tag="xh_sb")
        nc.sync.dma_start(xh_sb[:], xh8[:, :])

        ones8 = const.tile([P, 2, 16], F8, tag="ones8")
        nc.vector.memset(ones8[:], 1.0)
        junk8 = const.tile([P, 2, FD], F8, tag="junk8")
        nc.vector.memset(junk8[:], 1.0)
        rs_pb = const.tile([P, BT], F32, tag="rs_pb")
        r1_sb = const.tile([P, BT], F32, tag="r1_sb")
        sc_sb = const.tile([P, BT], F32, tag="sc_sb")

        acc = [
            accp.tile([P, NO, FD], F32, name=f"acc_{bt}", tag=f"acc_{bt}")
            for bt in range(BT)
        ]
        z14p = ctx.enter_context(tc.tile_pool(name="z14p", bufs=2))

        w_sb = {}

        def fetch_pair(pr, chunked):
            n0, n1 = pr * 2, pr * 2 + 1
            for n in (n0, n1):
                w_sb[n] = wpool.tile(
                    [P, KT2, 2, OUT_DIM], F8, name=f"w_{n}", tag="w_sb"
                )
            if chunked:
                # interleave the experts' K-chunks: the kt loop needs both
                # experts' chunk kt before it can proceed
                for kt in range(KT2):
                    for n in (n0, n1):
                        nc.sync.dma_start(w_sb[n][:, kt], w8[n, :, :][:, kt])
            else:
                for n in (n0, n1):
                    nc.sync.dma_start(w_sb[n][:], w8[n, :, :])

        fetch_pair(0, chunked=True)
        fetch_pair(1, chunked=True)
        fetch_pair(2, chunked=True)

        xb_sb = const.tile([P, BT, IN_DIM], BF16, tag="xb_sb")

        nc.vector.tensor_reduce(
            sc_sb[:], c_sb[:], axis=mybir.AxisListType.X, op=mybir.AluOpType.add
        )

        # --- PE clock warm-up: keep the PE busy through the DMA window
        # so the seeds and first main matmuls run at full p-state.
        jk = psum.tile([P, 2, NO, FD], F32, name="junk", tag="zp")
        for _ in range(10):
            nc.tensor.matmul(
                jk[0:1, 0, 0, :],
                lhsT=ones8[:, :, 0:1],
                rhs=junk8[:],
                start=True,
                stop=True,
                perf_mode=DR,
            )

        # --- bias seed: pt = (c @ bias) per bt, K=16 bf16 matmuls -------
        # Runs in the startup DMA window while xh/w0 stream in.
        seed_pt = []
        for half in range(2):
            pt = psum.tile([P, 2, NO, FD], F32, name=f"seed_{half}", tag="zp")
            for e in range(2):
                bt = half * 2 + e
                for ot in range(NO):
                    nc.tensor.matmul(
                        pt[:, e, ot],
                        lhsT=cT_sb[:, bt * P : (bt + 1) * P],
                        rhs=bias_sb[:, ot * FD : (ot + 1) * FD],
                        start=True,
                        stop=True,
                    )
            seed_pt.append(pt)
        for bt in range(BT):
            if bt % 2 == 0:
                nc.vector.tensor_copy(acc[bt][:], seed_pt[bt // 2][:, bt % 2])
            else:
                nc.scalar.activation(
                    acc[bt][:],
                    seed_pt[bt // 2][:, bt % 2],
                    mybir.ActivationFunctionType.Copy,
                )

        # --- main expert loop: pairs for 0-13, solo for 14/15 ----------
        # The two solo phases at the end spread the final combine+relu+
        # store chains over the last two expert windows instead of piling
        # all four behind the very last matmuls (which starved PSUM slots
        # and stalled the PE).
        groups = [(2 * p, 2 * p + 1) for p in range(NPAIR - 1)] + [(14,), (15,)]
        out_ap = out[:, :]
        for gi, grp in enumerate(groups):
            for bt in range(BT):
                ne = len(grp)
                zp = psum.tile([P, ne, NO, FD], F32, name="zp", tag="zp")
                for kt in range(KT2):
                    for ei, n in enumerate(grp):
                        for ot in range(NO):
                            nc.tensor.matmul(
                                zp[:, ei, ot],
                                lhsT=xh_sb[:, kt, :, bt * P : (bt + 1) * P],
                                rhs=w_sb[n][:, kt, :, ot * FD : (ot + 1) * FD],
                                start=(kt == 0),
                                stop=(kt == KT2 - 1),
                                perf_mode=DR,
                            )
                for ei, n in enumerate(grp):
                    if n == N_EXP - 2 and len(grp) == 1:
                        # solo expert 14: Scalar scales out of PSUM, GpSimd
                        # accumulates -- Vector stays free for expert 15
                        z14c = z14p.tile([P, NO, FD], F32, name="z14c", tag="z14c")
                        if bt % 2 == 0:
                            nc.scalar.activation(
                                z14c[:],
                                zp[:, ei],
                                mybir.ActivationFunctionType.Copy,
                                scale=c_sb[:, bt, n : n + 1],
                            )
                        else:
                            nc.vector.tensor_scalar(
                                out=z14c[:],
                                in0=zp[:, ei],
                                scalar1=c_sb[:, bt, n : n + 1],
                                scalar2=None,
                                op0=mybir.AluOpType.mult,
                            )
                        nc.gpsimd.tensor_tensor(
                            out=acc[bt][:],
                            in0=z14c[:],
                            in1=acc[bt][:],
                            op=mybir.AluOpType.add,
                        )
                    elif n != N_EXP - 1:
                        nc.vector.scalar_tensor_tensor(
                            out=acc[bt][:],
                            in0=zp[:, ei],
                            scalar=c_sb[:, bt, n : n + 1],
                            in1=acc[bt][:],
                            op0=mybir.AluOpType.mult,
                            op1=mybir.AluOpType.add,
                        )
                    else:
                        # last expert: combine + relu(+rank-1) + store per ot
                        for ot in range(NO):
                            nc.vector.scalar_tensor_tensor(
                                out=acc[bt][:, ot],
                                in0=zp[:, ei, ot],
                                scalar=c_sb[:, bt, n : n + 1],
                                in1=acc[bt][:, ot],
                                op0=mybir.AluOpType.mult,
                                op1=mybir.AluOpType.add,
                            )
                            nc.scalar.activation(
                                acc[bt][:, ot],
                                acc[bt][:, ot],
                                mybir.ActivationFunctionType.Relu,
                                bias=r1_sb[:, bt : bt + 1],
                            )
                            nc.sync.dma_start(
                                out_ap[:, bt, ot * FD : (ot + 1) * FD],
                                acc[bt][:, ot],
                            )

            if gi == 2:
                # rowsum(x) on DVE from the bf16 batch-major copy (off the
                # PE queue entirely; DMA'd here because the front half of
                # the kernel saturates the HBM wire with weights); r1 is
                # consumed only by the final ReLU bias.
                nc.sync.dma_start(xb_sb[:], xb16[:, :])
                nc.vector.tensor_reduce(
                    rs_pb[:],
                    xb_sb[:],
                    axis=mybir.AxisListType.X,
                    op=mybir.AluOpType.add,
                )
                # r1 = 0.5 * rowsum * sum_c
                nc.vector.scalar_tensor_tensor(
                    out=r1_sb[:],
                    in0=rs_pb[:],
                    scalar=0.5,
                    in1=sc_sb[:],
                    op0=mybir.AluOpType.mult,
                    op1=mybir.AluOpType.mult,
                )

            # prefetch two groups ahead: emitted after this group's
            # matmuls so the pool-slot WAR dependency sees the readers.
            if gi + 3 < len(groups):
                nxt = groups[gi + 3]
                for n in nxt:
                    w_sb[n] = wpool.tile(
                        [P, KT2, 2, OUT_DIM], F8, name=f"w_{n}", tag="w_sb"
                    )
                for kt in range(KT2):
                    for n in nxt:
                        nc.sync.dma_start(w_sb[n][:, kt], w8[n, :, :][:, kt])

    nc.compile()
    return nc


_NC_CACHE = {}


def _get_nc():
    if "nc" not in _NC_CACHE:
        _NC_CACHE["nc"] = _build_kernel()
    return _NC_CACHE["nc"]


def _xt_layout(x8):
    # fp8 [BS, IN_DIM] -> lhsT [P, KT2, 2, BS] with k = kt2*256+slot*128+p
    xT = np.ascontiguousarray(x8.T)  # [IN_DIM, BS]
    return np.ascontiguousarray(xT.reshape(KT2, 2, P, BS).transpose(2, 0, 1, 3))


def prepare_inputs(x, comp_weight, weight, bias):
    x = np.ascontiguousarray(np.asarray(x, dtype=np.float32))
    comp_weight = np.ascontiguousarray(np.asarray(comp_weight, dtype=np.float32))
    weight = np.asarray(weight, dtype=np.float32)
    bias = np.ascontiguousarray(np.asarray(bias, dtype=np.float32))

    # w = 0.5 + v; ship v in fp8 laid out [n, p, kt2, slot, o]
    v8 = (weight - np.float32(0.5)).astype(E4NP)
    w8 = np.ascontiguousarray(
        v8.reshape(N_EXP, KT2, 2, P, OUT_DIM).transpose(0, 3, 1, 2, 4)
    )
    bias_bf = bias.astype(ml_dtypes.bfloat16)

    in_maps = []
    for r in range(N_CORES):
        sl = slice(r * BS, (r + 1) * BS)
        xs = x[sl]
        cs = comp_weight[sl]
        xh = xs.astype(E4NP)
        in_maps.append(
            {
                "xh8": _xt_layout(xh),
                "xb16": np.ascontiguousarray(
                    xs.reshape(BT, P, IN_DIM).transpose(1, 0, 2)
                ).astype(ml_dtypes.bfloat16),
                "w8": w8,
                "c": np.ascontiguousarray(cs.reshape(BT, P, N_EXP).transpose(1, 0, 2)),
                "cT": np.ascontiguousarray(cs.T).astype(ml_dtypes.bfloat16),
                "bias": bias_bf,
            }
        )
    return in_maps


def _run(x, comp_weight, weight, bias, trace=False):
    in_maps = prepare_inputs(x, comp_weight, weight, bias)
    res = run_bass_kernel_spmd(
        _get_nc(), in_maps, core_ids=list(range(N_CORES)), trace=trace
    )
    out = np.concatenate(
        [
            res.results[r]["out"].transpose(1, 0, 2).reshape(BS, OUT_DIM)
            for r in range(N_CORES)
        ],
        axis=0,
    )
    return out, res


def kernel(x, comp_weight, weight, bias):
    out, _ = _run(x, comp_weight, weight, bias)
    return out


# revision 15
# speedup vs baseline: 1.0233x; 1.0233x over previous
"""Trainium2 Bass kernel for CompositionalFC (moe_routing).

Reference computation:
    z[n,b,o] = x[b,i] @ weight[n,i,o] + bias[n,o]
    out[b,o] = relu( sum_n comp_weight[b,n] * z[n,b,o] )

Strategy: data-parallel over batch across 8 NeuronCores (512 rows each,
weight/bias replicated), with the expert matmuls in fp8e4 DoubleRow mode
(2 contraction rows per PE pass = 2x bf16 matmul throughput, and half the
weight DMA traffic). Steady state measured at 216 ns per 512-col DoubleRow
matmul == the fp8 PE roofline (~157 TF/s effective per core).

Accuracy: fp8e4 has a 3-bit mantissa, too coarse for w ~ U[0,1) directly
(~3.4% rel err vs the 2e-2 gate). Mean-centering fixes it: w = 0.5 + v
with v ~ U[-.5,.5); quantize v to fp8 and add the exact rank-1 term
    0.5 * rowsum(x)[b] * (sum_n c[b,n]),
which also dominates the output magnitude. x ships as fp8 pair
x = xh + xl; the main pass uses xh only, while rowsum(x) is recovered as
rowsum(xh) + rowsum(xl) on device via ones-stationary DoubleRow matmuls
(single LdWeights, output [1, 512] transposed to [128, 4] by small
SBUF->SBUF DMAs). Measured end-to-end l2 rel err: 7.3e-3.

Per core: z_n accumulates in PSUM over 4 DoubleRow K-tiles of 256, then
one fused combine per expert on the Vector engine: acc = z*c[:,n] + acc.
The bias term (comp_weight @ bias) seeds the accumulators via K=16 bf16
matmuls, hidden in the startup DMA window behind PE-clock warm-up junk
matmuls; the rank-1 term is folded into the final ReLU's per-partition
bias on the Scalar engine.

Engine budget: Vector runs the full combines (~77% busy at steady state,
~2 us slack per expert group -- nothing big may sit in its queue);
Scalar (PSUM-capable) takes half the accumulator seeds, half of expert
14's scale-copies, the rowsum drain, and the final ReLUs; GpSimd
(SBUF-only on TRN2) takes expert 14's accumulate-adds and the r1
compute. Expert groups: pairs for 0-13 sharing each stationary xh tile
across 2 experts x 2 PSUM banks (4 matmuls per LdWeights), then experts
14 and 15 solo so the final drain chains spread over the last two expert
windows instead of piling up behind the very last matmuls. Weight pair
DMAs interleave the two experts' K-chunks (the kt loop needs both
experts' chunk kt first); later groups prefetch two ahead through a
4-deep pool.
"""

import sys

for _p in ("/opt/trn_rl_repo",):
    if _p not in sys.path:
        sys.path.insert(0, _p)

from contextlib import ExitStack

import ml_dtypes
import numpy as np

import concourse.bass as bass
import concourse.mybir as mybir
import concourse.tile as tile
from concourse import bacc
from concourse.bass_utils import run_bass_kernel_spmd

N_CORES = 8
BATCH, IN_DIM, OUT_DIM, N_EXP = 4096, 1024, 1024, 16
BS = BATCH // N_CORES          # 512 batch rows per core
P = 128                        # partitions
BT = BS // P                   # 4 batch tiles per core
KT2 = IN_DIM // 256            # 4 DoubleRow contraction tiles (K=256 each)
FD = 512                       # matmul free dim / PSUM bank width (fp32)
NO = OUT_DIM // FD             # 2 output column tiles
NPAIR = N_EXP // 2

F32 = mybir.dt.float32
BF16 = mybir.dt.bfloat16
F8 = mybir.dt.float8e4
DR = mybir.MatmulPerfMode.DoubleRow
ACT = mybir.ActivationFunctionType

E4NP = ml_dtypes.float8_e4m3   # TRN fp8e4 == IEEE e4m3 (max 240)


def _build_kernel():
    nc = bacc.Bacc(
        "TRN2",
        target_bir_lowering=False,
        debug=False,
        num_devices=N_CORES,
    )
    # k = kt2*256 + slot*128 + p; b = bt*128 + p_out
    xh8 = nc.declare_dram_parameter("xh8", [P, KT2, 2, BS], F8, isOutput=False)
    xl8 = nc.declare_dram_parameter("xl8", [P, KT2, 2, BS], F8, isOutput=False)
    w8 = nc.declare_dram_parameter("w8", [N_EXP, P, KT2, 2, OUT_DIM], F8, isOutput=False)
    c = nc.declare_dram_parameter("c", [P, BT, N_EXP], F32, isOutput=False)
    cT = nc.declare_dram_parameter("cT", [N_EXP, BS], BF16, isOutput=False)
    bias = nc.declare_dram_parameter("bias", [N_EXP, OUT_DIM], BF16, isOutput=False)
    out = nc.declare_dram_parameter("out", [P, BT, OUT_DIM], F32, isOutput=True)

    with ExitStack() as ctx:
        tc = ctx.enter_context(tile.TileContext(nc))
        const = ctx.enter_context(tc.tile_pool(name="const", bufs=1))
        accp = ctx.enter_context(tc.tile_pool(name="accp", bufs=1))
        wpool = ctx.enter_context(tc.tile_pool(name="wpool", bufs=4))
        z14p = ctx.enter_context(tc.tile_pool(name="z14p", bufs=2))
        psum = ctx.enter_context(tc.tile_pool(name="psum", bufs=2, space="PSUM"))

        # --- startup DMAs, all on sync (GpSimd's DMA path has ~5us
        # completion latency), most-gating first ------------------------
        cT_sb = const.tile([N_EXP, BS], BF16, tag="cT_sb")
        nc.sync.dma_start(cT_sb[:], cT[:, :])
        bias_sb = const.tile([N_EXP, OUT_DIM], BF16, tag="bias_sb")
        nc.sync.dma_start(bias_sb[:], bias[:, :])
        c_sb = const.tile([P, BT, N_EXP], F32, tag="c_sb")
        nc.sync.dma_start(c_sb[:], c[:, :])
        xh_sb = const.tile([P, KT2, 2, BS], F8, tag="xh_sb")
        nc.sync.dma_start(xh_sb[:], xh8[:, :])

        ones8 = const.tile([P, 2, 16], F8, tag="ones8")
        nc.vector.memset(ones8[:], 1.0)
        junk8 = const.tile([P, 2, FD], F8, tag="junk8")
        nc.vector.memset(junk8[:], 1.0)
        rs_row = const.tile([1, BS], F32, tag="rs_row")
        rs_pb = const.tile([P, BT], F32, tag="rs_pb")
        r1_sb = const.tile([P, BT], F32, tag="r1_sb")
        sc_sb = const.tile([P, BT], F32, tag="sc_sb")

        acc = [
            accp.tile([P, NO, FD], F32, name=f"acc_{bt}", tag=f"acc_{bt}")
            for bt in range(BT)
        ]

        w_sb = {}

        def fetch_group(grp, chunked):
            for n in grp:
                w_sb[n] = wpool.tile(
                    [P, KT2, 2, OUT_DIM], F8, name=f"w_{n}", tag="w_sb"
                )
            if chunked:
                # interleave the experts' K-chunks: the kt loop needs both
                # experts' chunk kt before it can proceed
                for kt in range(KT2):
                    for n in grp:
                        nc.sync.dma_start(w_sb[n][:, kt], w8[n, :, :][:, kt])
            else:
                for n in grp:
                    nc.sync.dma_start(w_sb[n][:], w8[n, :, :])

        groups = [(2 * p, 2 * p + 1) for p in range(NPAIR - 1)] + [(14,), (15,)]
        fetch_group(groups[0], chunked=True)

        xl_sb = const.tile([P, KT2, 2, BS], F8, tag="xl_sb")
        nc.sync.dma_start(xl_sb[:], xl8[:, :])

        fetch_group(groups[1], chunked=True)

        nc.vector.tensor_reduce(
            sc_sb[:], c_sb[:], axis=mybir.AxisListType.X, op=mybir.AluOpType.add
        )

        # --- PE clock warm-up: keep the PE busy through the DMA window
        # so the seeds and first main matmuls run at full p-state.
        jk = psum.tile([P, 2, NO, FD], F32, name="junk", tag="zp")
        for _ in range(10):
            nc.tensor.matmul(
                jk[0:1, 0, 0, :],
                lhsT=ones8[:, :, 0:1],
                rhs=junk8[:],
                start=True,
                stop=True,
                perf_mode=DR,
            )

        # --- bias seed: pt = (c @ bias) per bt, K=16 bf16 matmuls -------
        # Runs in the startup DMA window while xh/w0 stream in. The
        # accumulator-init copies split across Vector and Scalar so the
        # seed-tile WAR (which gates the first main matmuls' PSUM slots)
        # clears right after the seeds.
        seed_pt = []
        for half in range(2):
            pt = psum.tile([P, 2, NO, FD], F32, name=f"seed_{half}", tag="zp")
            for e in range(2):
                bt = half * 2 + e
                for ot in range(NO):
                    nc.tensor.matmul(
                        pt[:, e, ot],
                        lhsT=cT_sb[:, bt * P : (bt + 1) * P],
                        rhs=bias_sb[:, ot * FD : (ot + 1) * FD],
                        start=True,
                        stop=True,
                    )
            seed_pt.append(pt)
        for bt in range(BT):
            if bt % 2 == 0:
                nc.vector.tensor_copy(acc[bt][:], seed_pt[bt // 2][:, bt % 2])
            else:
                nc.scalar.activation(
                    acc[bt][:], seed_pt[bt // 2][:, bt % 2], ACT.Copy
                )

        # --- main expert loop: pairs for 0-13, solo for 14/15 ----------
        out_ap = out[:, :]
        for gi, grp in enumerate(groups):
            for bt in range(BT):
                ne = len(grp)
                zp = psum.tile([P, ne, NO, FD], F32, name="zp", tag="zp")
                for kt in range(KT2):
                    for ei, n in enumerate(grp):
                        for ot in range(NO):
                            nc.tensor.matmul(
                                zp[:, ei, ot],
                                lhsT=xh_sb[:, kt, :, bt * P : (bt + 1) * P],
                                rhs=w_sb[n][:, kt, :, ot * FD : (ot + 1) * FD],
                                start=(kt == 0),
                                stop=(kt == KT2 - 1),
                                perf_mode=DR,
                            )
                for ei, n in enumerate(grp):
                    if n == N_EXP - 2:
                        # solo expert 14: Scalar/Vector alternate the
                        # scale-out-of-PSUM; GpSimd accumulates. Keeps the
                        # Vector queue clear for expert 15's final chains.
                        z14c = z14p.tile([P, NO, FD], F32, name="z14c", tag="z14c")
                        if bt % 2 == 0:
                            nc.scalar.activation(
                                z14c[:],
                                zp[:, ei],
                                ACT.Copy,
                                scale=c_sb[:, bt, n : n + 1],
                            )
                        else:
                            nc.vector.tensor_scalar(
                                out=z14c[:],
                                in0=zp[:, ei],
                                scalar1=c_sb[:, bt, n : n + 1],
                                scalar2=None,
                                op0=mybir.AluOpType.mult,
                            )
                        nc.gpsimd.tensor_tensor(
                            out=acc[bt][:],
                            in0=z14c[:],
                            in1=acc[bt][:],
                            op=mybir.AluOpType.add,
                        )
                    elif n != N_EXP - 1:
                        nc.vector.scalar_tensor_tensor(
                            out=acc[bt][:],
                            in0=zp[:, ei],
                            scalar=c_sb[:, bt, n : n + 1],
                            in1=acc[bt][:],
                            op0=mybir.AluOpType.mult,
                            op1=mybir.AluOpType.add,
                        )
                    else:
                        # last expert: combine + relu(+rank-1 bias) + store
                        for ot in range(NO):
                            nc.vector.scalar_tensor_tensor(
                                out=acc[bt][:, ot],
                                in0=zp[:, ei, ot],
                                scalar=c_sb[:, bt, n : n + 1],
                                in1=acc[bt][:, ot],
                                op0=mybir.AluOpType.mult,
                                op1=mybir.AluOpType.add,
                            )
                            nc.scalar.activation(
                                acc[bt][:, ot],
                                acc[bt][:, ot],
                                ACT.Relu,
                                bias=r1_sb[:, bt : bt + 1],
                            )
                            nc.sync.dma_start(
                                out_ap[:, bt, ot * FD : (ot + 1) * FD],
                                acc[bt][:, ot],
                            )

            if gi == 0:
                # --- rowsum(x) = rowsum(xh) + rowsum(xl) ---------------
                # ones-stationary DoubleRow matmuls -> [1, 512] on
                # partition 0, transposed to [128, 4] via small DMAs.
                # Emitted after group 0 so it never gates the startup;
                # drained by Scalar and combined on GpSimd so the Vector
                # combine stream is untouched; r1 is consumed only by the
                # final ReLU bias.
                rs_pt = psum.tile([P, 2, NO, FD], F32, name="rs", tag="zp")
                n_rs = 2 * KT2
                i_rs = 0
                for xt in (xh_sb, xl_sb):
                    for kt in range(KT2):
                        nc.tensor.matmul(
                            rs_pt[0:1, 0, 0, :],
                            lhsT=ones8[:, :, 0:1],
                            rhs=xt[:, kt],
                            start=(i_rs == 0),
                            stop=(i_rs == n_rs - 1),
                            perf_mode=DR,
                        )
                        i_rs += 1
                nc.scalar.activation(rs_row[:], rs_pt[0:1, 0, 0, :], ACT.Copy)
                for bt in range(BT):
                    nc.sync.dma_start(
                        rs_pb[:, bt : bt + 1], rs_row[0:1, bt * P : (bt + 1) * P]
                    )
                # r1 = 0.5 * rowsum * sum_c   (tiny op, [128, 4])
                nc.vector.scalar_tensor_tensor(
                    out=r1_sb[:],
                    in0=rs_pb[:],
                    scalar=0.5,
                    in1=sc_sb[:],
                    op0=mybir.AluOpType.mult,
                    op1=mybir.AluOpType.mult,
                )

            # prefetch two groups ahead: emitted after this group's
            # matmuls so the pool-slot WAR dependency sees the readers.
            if gi + 2 < len(groups):
                fetch_group(groups[gi + 2], chunked=False)

    nc.compile()
    return nc


_NC_CACHE = {}


def _get_nc():
    if "nc" not in _NC_CACHE:
        _NC_CACHE["nc"] = _build_kernel()
    return _NC_CACHE["nc"]


def _xt_layout(x8):
    # fp8 [BS, IN_DIM] -> lhsT [P, KT2, 2, BS] with k = kt2*256+slot*128+p
    xT = np.ascontiguousarray(x8.T)  # [IN_DIM, BS]
    return np.ascontiguousarray(xT.reshape(KT2, 2, P, BS).transpose(2, 0, 1, 3))


def prepare_inputs(x, comp_weight, weight, bias):
    x = np.ascontiguousarray(np.asarray(x, dtype=np.float32))
    comp_weight = np.ascontiguousarray(np.asarray(comp_weight, dtype=np.float32))
    weight = np.asarray(weight, dtype=np.float32)
    bias = np.ascontiguousarray(np.asarray(bias, dtype=np.float32))

    # w = 0.5 + v; ship v in fp8 laid out [n, p, kt2, slot, o]
    v8 = (weight - np.float32(0.5)).astype(E4NP)
    w8 = np.ascontiguousarray(
        v8.reshape(N_EXP, KT2, 2, P, OUT_DIM).transpose(0, 3, 1, 2, 4)
    )
    bias_bf = bias.astype(ml_dtypes.bfloat16)

    in_maps = []
    for r in range(N_CORES):
        sl = slice(r * BS, (r + 1) * BS)
        xs = x[sl]
        cs = comp_weight[sl]
        xh = xs.astype(E4NP)
        xl = (xs - xh.astype(np.float32)).astype(E4NP)
        in_maps.append(
            {
                "xh8": _xt_layout(xh),
                "xl8": _xt_layout(xl),
                "w8": w8,
                "c": np.ascontiguousarray(cs.reshape(BT, P, N_EXP).transpose(1, 0, 2)),
                "cT": np.ascontiguousarray(cs.T).astype(ml_dtypes.bfloat16),
                "bias": bias_bf,
            }
        )
    return in_maps


def _run(x, comp_weight, weight, bias, trace=False):
    in_maps = prepare_inputs(x, comp_weight, weight, bias)
    res = run_bass_kernel_spmd(
        _get_nc(), in_maps, core_ids=list(range(N_CORES)), trace=trace
    )
    out = np.concatenate(
        [
            res.results[r]["out"].transpose(1, 0, 2).reshape(BS, OUT_DIM)
            for r in range(N_CORES)
        ],
        axis=0,
    )
    return out, res


def kernel(x, comp_weight, weight, bias):
    out, _ = _run(x, comp_weight, weight, bias)
    return out


# revision 16
# speedup vs baseline: 1.0305x; 1.0070x over previous
"""Trainium2 Bass kernel for CompositionalFC (moe_routing).

Reference computation:
    z[n,b,o] = x[b,i] @ weight[n,i,o] + bias[n,o]
    out[b,o] = relu( sum_n comp_weight[b,n] * z[n,b,o] )

Strategy: data-parallel over batch across 8 NeuronCores (512 rows each,
weight/bias replicated), with the expert matmuls in fp8e4 DoubleRow mode
(2 contraction rows per PE pass = 2x bf16 matmul throughput, and half the
weight DMA traffic). Steady state measured at 216 ns per 512-col DoubleRow
matmul == the fp8 PE roofline (~157 TF/s effective per core).

Accuracy: fp8e4 has a 3-bit mantissa, too coarse for w ~ U[0,1) directly
(~3.4% rel err vs the 2e-2 gate). Mean-centering fixes it: w = 0.5 + v
with v ~ U[-.5,.5); quantize v to fp8 and add the exact rank-1 term
    0.5 * rowsum(x)[b] * (sum_n c[b,n]),
which also dominates the output magnitude. x ships as fp8 pair
x = xh + xl; the main pass uses xh only, while rowsum(x) is recovered as
rowsum(xh) + rowsum(xl) on device via ones-stationary DoubleRow matmuls
(single LdWeights, output [1, 512] transposed to [128, 4] by small
SBUF->SBUF DMAs). Measured end-to-end l2 rel err: 7.3e-3.

Per core: z_n accumulates in PSUM over 4 DoubleRow K-tiles of 256, then
one fused combine per expert on the Vector engine: acc = z*c[:,n] + acc.
The bias term (comp_weight @ bias) seeds the accumulators via K=16 bf16
matmuls, hidden in the startup DMA window behind PE-clock warm-up junk
matmuls; the rank-1 term is folded into the final ReLU's per-partition
bias on the Scalar engine.

Engine budget: Vector runs the full combines (~77% busy at steady state,
~2 us slack per expert group -- nothing big may sit in its queue);
Scalar (PSUM-capable) takes half the accumulator seeds, the rowsum
drain, and the final ReLUs. GpSimd is unused for tensor work (no PSUM
access on TRN2 and its tensor ops run at half Vector rate).
Expert groups: pairs for 0-13 sharing each stationary xh tile
across 2 experts x 2 PSUM banks (4 matmuls per LdWeights), then experts
14 and 15 solo so the final drain chains spread over the last two expert
windows instead of piling up behind the very last matmuls. Weight pair
DMAs interleave the two experts' K-chunks (the kt loop needs both
experts' chunk kt first); later groups prefetch two ahead through a
4-deep pool.
"""

import sys

for _p in ("/opt/trn_rl_repo",):
    if _p not in sys.path:
        sys.path.insert(0, _p)

from contextlib import ExitStack

import ml_dtypes
import numpy as np

import concourse.bass as bass
import concourse.mybir as mybir
import concourse.tile as tile
from concourse import bacc
from concourse.bass_utils import run_bass_kernel_spmd

N_CORES = 8
BATCH, IN_DIM, OUT_DIM, N_EXP = 4096, 1024, 1024, 16
BS = BATCH // N_CORES          # 512 batch rows per core
P = 128                        # partitions
BT = BS // P                   # 4 batch tiles per core
KT2 = IN_DIM // 256            # 4 DoubleRow contraction tiles (K=256 each)
FD = 512                       # matmul free dim / PSUM bank width (fp32)
NO = OUT_DIM // FD             # 2 output column tiles
NPAIR = N_EXP // 2

F32 = mybir.dt.float32
BF16 = mybir.dt.bfloat16
F8 = mybir.dt.float8e4
DR = mybir.MatmulPerfMode.DoubleRow
ACT = mybir.ActivationFunctionType

E4NP = ml_dtypes.float8_e4m3   # TRN fp8e4 == IEEE e4m3 (max 240)


def _build_kernel():
    nc = bacc.Bacc(
        "TRN2",
        target_bir_lowering=False,
        debug=False,
        num_devices=N_CORES,
    )
    # k = kt2*256 + slot*128 + p; b = bt*128 + p_out
    xh8 = nc.declare_dram_parameter("xh8", [P, KT2, 2, BS], F8, isOutput=False)
    xl8 = nc.declare_dram_parameter("xl8", [P, KT2, 2, BS], F8, isOutput=False)
    w8 = nc.declare_dram_parameter("w8", [N_EXP, P, KT2, 2, OUT_DIM], F8, isOutput=False)
    c = nc.declare_dram_parameter("c", [P, BT, N_EXP], F32, isOutput=False)
    cT = nc.declare_dram_parameter("cT", [N_EXP, BS], BF16, isOutput=False)
    bias = nc.declare_dram_parameter("bias", [N_EXP, OUT_DIM], BF16, isOutput=False)
    out = nc.declare_dram_parameter("out", [P, BT, OUT_DIM], F32, isOutput=True)

    with ExitStack() as ctx:
        tc = ctx.enter_context(tile.TileContext(nc))
        const = ctx.enter_context(tc.tile_pool(name="const", bufs=1))
        accp = ctx.enter_context(tc.tile_pool(name="accp", bufs=1))
        wpool = ctx.enter_context(tc.tile_pool(name="wpool", bufs=4))
        psum = ctx.enter_context(tc.tile_pool(name="psum", bufs=2, space="PSUM"))

        # --- startup DMAs, all on sync (GpSimd's DMA path has ~5us
        # completion latency), most-gating first ------------------------
        cT_sb = const.tile([N_EXP, BS], BF16, tag="cT_sb")
        nc.sync.dma_start(cT_sb[:], cT[:, :])
        bias_sb = const.tile([N_EXP, OUT_DIM], BF16, tag="bias_sb")
        nc.sync.dma_start(bias_sb[:], bias[:, :])
        c_sb = const.tile([P, BT, N_EXP], F32, tag="c_sb")
        nc.sync.dma_start(c_sb[:], c[:, :])
        xh_sb = const.tile([P, KT2, 2, BS], F8, tag="xh_sb")
        nc.sync.dma_start(xh_sb[:], xh8[:, :])

        ones8 = const.tile([P, 2, 16], F8, tag="ones8")
        nc.vector.memset(ones8[:], 1.0)
        junk8 = const.tile([P, 2, FD], F8, tag="junk8")
        nc.vector.memset(junk8[:], 1.0)
        rs_row = const.tile([1, BS], F32, tag="rs_row")
        rs_pb = const.tile([P, BT], F32, tag="rs_pb")
        r1_sb = const.tile([P, BT], F32, tag="r1_sb")
        sc_sb = const.tile([P, BT], F32, tag="sc_sb")

        acc = [
            accp.tile([P, NO, FD], F32, name=f"acc_{bt}", tag=f"acc_{bt}")
            for bt in range(BT)
        ]

        w_sb = {}

        def fetch_group(grp, chunked):
            for n in grp:
                w_sb[n] = wpool.tile(
                    [P, KT2, 2, OUT_DIM], F8, name=f"w_{n}", tag="w_sb"
                )
            if chunked:
                # interleave the experts' K-chunks: the kt loop needs both
                # experts' chunk kt before it can proceed
                for kt in range(KT2):
                    for n in grp:
                        nc.sync.dma_start(w_sb[n][:, kt], w8[n, :, :][:, kt])
            else:
                for n in grp:
                    nc.sync.dma_start(w_sb[n][:], w8[n, :, :])

        groups = [(2 * p, 2 * p + 1) for p in range(NPAIR - 1)] + [(14,), (15,)]
        fetch_group(groups[0], chunked=True)

        xl_sb = const.tile([P, KT2, 2, BS], F8, tag="xl_sb")
        nc.sync.dma_start(xl_sb[:], xl8[:, :])

        fetch_group(groups[1], chunked=True)

        nc.vector.tensor_reduce(
            sc_sb[:], c_sb[:], axis=mybir.AxisListType.X, op=mybir.AluOpType.add
        )

        # --- PE clock warm-up: keep the PE busy through the DMA window
        # so the seeds and first main matmuls run at full p-state.
        jk = psum.tile([P, 2, NO, FD], F32, name="junk", tag="zp")
        for _ in range(10):
            nc.tensor.matmul(
                jk[0:1, 0, 0, :],
                lhsT=ones8[:, :, 0:1],
                rhs=junk8[:],
                start=True,
                stop=True,
                perf_mode=DR,
            )

        # --- bias seed: pt = (c @ bias) per bt, K=16 bf16 matmuls -------
        # Runs in the startup DMA window while xh/w0 stream in. The
        # accumulator-init copies split across Vector and Scalar so the
        # seed-tile WAR (which gates the first main matmuls' PSUM slots)
        # clears right after the seeds.
        seed_pt = []
        for half in range(2):
            pt = psum.tile([P, 2, NO, FD], F32, name=f"seed_{half}", tag="zp")
            for e in range(2):
                bt = half * 2 + e
                for ot in range(NO):
                    nc.tensor.matmul(
                        pt[:, e, ot],
                        lhsT=cT_sb[:, bt * P : (bt + 1) * P],
                        rhs=bias_sb[:, ot * FD : (ot + 1) * FD],
                        start=True,
                        stop=True,
                    )
            seed_pt.append(pt)
        for bt in range(BT):
            if bt % 2 == 0:
                nc.vector.tensor_copy(acc[bt][:], seed_pt[bt // 2][:, bt % 2])
            else:
                nc.scalar.activation(
                    acc[bt][:], seed_pt[bt // 2][:, bt % 2], ACT.Copy
                )

        # --- main expert loop: pairs for 0-13, solo for 14/15 ----------
        out_ap = out[:, :]
        for gi, grp in enumerate(groups):
            for bt in range(BT):
                ne = len(grp)
                zp = psum.tile([P, ne, NO, FD], F32, name="zp", tag="zp")
                if grp == (N_EXP - 1,):
                    # ot-major: close each ot's accumulation group early so
                    # the final combine/relu/store overlaps ot1's streaming
                    mm_order = [
                        (kt, 0, ot) for ot in range(NO) for kt in range(KT2)
                    ]
                else:
                    mm_order = [
                        (kt, ei, ot)
                        for kt in range(KT2)
                        for ei in range(ne)
                        for ot in range(NO)
                    ]
                for kt, ei, ot in mm_order:
                    nc.tensor.matmul(
                        zp[:, ei, ot],
                        lhsT=xh_sb[:, kt, :, bt * P : (bt + 1) * P],
                        rhs=w_sb[grp[ei]][:, kt, :, ot * FD : (ot + 1) * FD],
                        start=(kt == 0),
                        stop=(kt == KT2 - 1),
                        perf_mode=DR,
                    )
                for ei, n in enumerate(grp):
                    if n != N_EXP - 1:
                        nc.vector.scalar_tensor_tensor(
                            out=acc[bt][:],
                            in0=zp[:, ei],
                            scalar=c_sb[:, bt, n : n + 1],
                            in1=acc[bt][:],
                            op0=mybir.AluOpType.mult,
                            op1=mybir.AluOpType.add,
                        )
                    else:
                        # last expert: combine + relu(+rank-1 bias) + store
                        for ot in range(NO):
                            nc.vector.scalar_tensor_tensor(
                                out=acc[bt][:, ot],
                                in0=zp[:, ei, ot],
                                scalar=c_sb[:, bt, n : n + 1],
                                in1=acc[bt][:, ot],
                                op0=mybir.AluOpType.mult,
                                op1=mybir.AluOpType.add,
                            )
                            nc.scalar.activation(
                                acc[bt][:, ot],
                                acc[bt][:, ot],
                                ACT.Relu,
                                bias=r1_sb[:, bt : bt + 1],
                            )
                            nc.sync.dma_start(
                                out_ap[:, bt, ot * FD : (ot + 1) * FD],
                                acc[bt][:, ot],
                            )

            if gi == 0:
                # --- rowsum(x) = rowsum(xh) + rowsum(xl) ---------------
                # ones-stationary DoubleRow matmuls -> [1, 512] on
                # partition 0, transposed to [128, 4] via small DMAs.
                # Emitted after group 0 so it never gates the startup;
                # drained by Scalar and combined on GpSimd so the Vector
                # combine stream is untouched; r1 is consumed only by the
                # final ReLU bias.
                rs_pt = psum.tile([P, 2, NO, FD], F32, name="rs", tag="zp")
                n_rs = 2 * KT2
                i_rs = 0
                for xt in (xh_sb, xl_sb):
                    for kt in range(KT2):
                        nc.tensor.matmul(
                            rs_pt[0:1, 0, 0, :],
                            lhsT=ones8[:, :, 0:1],
                            rhs=xt[:, kt],
                            start=(i_rs == 0),
                            stop=(i_rs == n_rs - 1),
                            perf_mode=DR,
                        )
                        i_rs += 1
                nc.scalar.activation(rs_row[:], rs_pt[0:1, 0, 0, :], ACT.Copy)
                for bt in range(BT):
                    nc.sync.dma_start(
                        rs_pb[:, bt : bt + 1], rs_row[0:1, bt * P : (bt + 1) * P]
                    )
                # r1 = 0.5 * rowsum * sum_c   (tiny op, [128, 4])
                nc.vector.scalar_tensor_tensor(
                    out=r1_sb[:],
                    in0=rs_pb[:],
                    scalar=0.5,
                    in1=sc_sb[:],
                    op0=mybir.AluOpType.mult,
                    op1=mybir.AluOpType.mult,
                )

            # prefetch two groups ahead: emitted after this group's
            # matmuls so the pool-slot WAR dependency sees the readers.
            if gi + 2 < len(groups):
                fetch_group(groups[gi + 2], chunked=False)

    nc.compile()
    return nc


_NC_CACHE = {}


def _get_nc():
    if "nc" not in _NC_CACHE:
        _NC_CACHE["nc"] = _build_kernel()
    return _NC_CACHE["nc"]


def _xt_layout(x8):
    # fp8 [BS, IN_DIM] -> lhsT [P, KT2, 2, BS] with k = kt2*256+slot*128+p
    xT = np.ascontiguousarray(x8.T)  # [IN_DIM, BS]
    return np.ascontiguousarray(xT.reshape(KT2, 2, P, BS).transpose(2, 0, 1, 3))


def prepare_inputs(x, comp_weight, weight, bias):
    x = np.ascontiguousarray(np.asarray(x, dtype=np.float32))
    comp_weight = np.ascontiguousarray(np.asarray(comp_weight, dtype=np.float32))
    weight = np.asarray(weight, dtype=np.float32)
    bias = np.ascontiguousarray(np.asarray(bias, dtype=np.float32))

    # w = 0.5 + v; ship v in fp8 laid out [n, p, kt2, slot, o]
    v8 = (weight - np.float32(0.5)).astype(E4NP)
    w8 = np.ascontiguousarray(
        v8.reshape(N_EXP, KT2, 2, P, OUT_DIM).transpose(0, 3, 1, 2, 4)
    )
    bias_bf = bias.astype(ml_dtypes.bfloat16)

    in_maps = []
    for r in range(N_CORES):
        sl = slice(r * BS, (r + 1) * BS)
        xs = x[sl]
        cs = comp_weight[sl]
        xh = xs.astype(E4NP)
        xl = (xs - xh.astype(np.float32)).astype(E4NP)
        in_maps.append(
            {
                "xh8": _xt_layout(xh),
                "xl8": _xt_layout(xl),
                "w8": w8,
                "c": np.ascontiguousarray(cs.reshape(BT, P, N_EXP).transpose(1, 0, 2)),
                "cT": np.ascontiguousarray(cs.T).astype(ml_dtypes.bfloat16),
                "bias": bias_bf,
            }
        )
    return in_maps


def _run(x, comp_weight, weight, bias, trace=False):
    in_maps = prepare_inputs(x, comp_weight, weight, bias)
    res = run_bass_kernel_spmd(
        _get_nc(), in_maps, core_ids=list(range(N_CORES)), trace=trace
    )
    out = np.concatenate(
        [
            res.results[r]["out"].transpose(1, 0, 2).reshape(BS, OUT_DIM)
            for r in range(N_CORES)
        ],
        axis=0,
    )
    return out, res


def kernel(x, comp_weight, weight, bias):
    out, _ = _run(x, comp_weight, weight, bias)
    return out


# revision 17
# speedup vs baseline: 1.0980x; 1.0655x over previous
"""Trainium2 Bass kernel for CompositionalFC (moe_routing).

Reference computation:
    z[n,b,o] = x[b,i] @ weight[n,i,o] + bias[n,o]
    out[b,o] = relu( sum_n comp_weight[b,n] * z[n,b,o] )

Strategy: data-parallel over batch across 8 NeuronCores (512 rows each,
weight/bias replicated), with the expert matmuls in fp8e4 DoubleRow mode
(2 contraction rows per PE pass = 2x bf16 matmul throughput, and half the
weight DMA traffic). Steady state measured at 216 ns per 512-col DoubleRow
matmul == the fp8 PE roofline (~157 TF/s effective per core).

Accuracy: fp8e4 has a 3-bit mantissa, too coarse for w ~ U[0,1) directly
(~3.4% rel err vs the 2e-2 gate). Mean-centering fixes it: w = 0.5 + v
with v ~ U[-.5,.5); quantize v to fp8 and add the exact rank-1 term
    0.5 * rowsum(x)[b] * (sum_n c[b,n]),
which also dominates the output magnitude. x ships as fp8 pair
x = xh + xl; the main pass uses xh only, while rowsum(x) is recovered as
rowsum(xh) + rowsum(xl) on device via ones-stationary DoubleRow matmuls
(single LdWeights, output [1, 512] transposed to [128, 4] by small
SBUF->SBUF DMAs). Measured end-to-end l2 rel err: 7.3e-3.

Per core: z_n accumulates in PSUM over 4 DoubleRow K-tiles of 256, then
one fused combine per expert on the Vector engine: acc = z*c[:,n] + acc.
The bias term (comp_weight @ bias) seeds the accumulators via K=16 bf16
matmuls, hidden in the startup DMA window behind PE-clock warm-up junk
matmuls; the rank-1 term is folded into the final ReLU's per-partition
bias on the Scalar engine.

Engine budget: Vector runs the full combines (~77% busy at steady state,
~2 us slack per expert group -- nothing big may sit in its queue);
Scalar (PSUM-capable) takes half the accumulator seeds, the rowsum
drain, and the final ReLUs. GpSimd is unused for tensor work (no PSUM
access on TRN2 and its tensor ops run at half Vector rate).
Expert groups: pairs for 0-13 sharing each stationary xh tile
across 2 experts x 2 PSUM banks (4 matmuls per LdWeights), then experts
14 and 15 solo so the final drain chains spread over the last two expert
windows instead of piling up behind the very last matmuls. Weight pair
DMAs interleave the two experts' K-chunks (the kt loop needs both
experts' chunk kt first); later groups prefetch two ahead through a
4-deep pool.
"""

import sys

for _p in ("/opt/trn_rl_repo",):
    if _p not in sys.path:
        sys.path.insert(0, _p)

from contextlib import ExitStack

import ml_dtypes
import numpy as np

import concourse.bass as bass
import concourse.mybir as mybir
import concourse.tile as tile
from concourse import bacc
from concourse.bass_utils import run_bass_kernel_spmd

N_CORES = 8
BATCH, IN_DIM, OUT_DIM, N_EXP = 4096, 1024, 1024, 16
BS = BATCH // N_CORES          # 512 batch rows per core
P = 128                        # partitions
BT = BS // P                   # 4 batch tiles per core
KT2 = IN_DIM // 256            # 4 DoubleRow contraction tiles (K=256 each)
FD = 512                       # matmul free dim / PSUM bank width (fp32)
NO = OUT_DIM // FD             # 2 output column tiles
NPAIR = N_EXP // 2

F32 = mybir.dt.float32
BF16 = mybir.dt.bfloat16
F8 = mybir.dt.float8e4
DR = mybir.MatmulPerfMode.DoubleRow
ACT = mybir.ActivationFunctionType

E4NP = ml_dtypes.float8_e4m3   # TRN fp8e4 == IEEE e4m3 (max 240)


def _build_kernel():
    nc = bacc.Bacc(
        "TRN2",
        target_bir_lowering=False,
        debug=False,
        num_devices=N_CORES,
    )
    # k = kt2*256 + slot*128 + p; b = bt*128 + p_out
    xh8 = nc.declare_dram_parameter("xh8", [P, KT2, 2, BS], F8, isOutput=False)
    xl8 = nc.declare_dram_parameter("xl8", [P, KT2, 2, BS], F8, isOutput=False)
    w8 = nc.declare_dram_parameter("w8", [N_EXP, P, KT2, 2, OUT_DIM], F8, isOutput=False)
    c = nc.declare_dram_parameter("c", [P, BT, N_EXP], F32, isOutput=False)
    cT = nc.declare_dram_parameter("cT", [N_EXP, BS], BF16, isOutput=False)
    bias = nc.declare_dram_parameter("bias", [N_EXP, OUT_DIM], BF16, isOutput=False)
    out = nc.declare_dram_parameter("out", [P, BT, OUT_DIM], F32, isOutput=True)

    with ExitStack() as ctx:
        tc = ctx.enter_context(tile.TileContext(nc))
        const = ctx.enter_context(tc.tile_pool(name="const", bufs=1))
        accp = ctx.enter_context(tc.tile_pool(name="accp", bufs=1))
        wpool = ctx.enter_context(tc.tile_pool(name="wpool", bufs=4))
        psum = ctx.enter_context(tc.tile_pool(name="psum", bufs=4, space="PSUM"))

        # --- startup DMAs, all on sync (GpSimd's DMA path has ~5us
        # completion latency), most-gating first ------------------------
        cT_sb = const.tile([N_EXP, BS], BF16, tag="cT_sb")
        nc.sync.dma_start(cT_sb[:], cT[:, :])
        bias_sb = const.tile([N_EXP, OUT_DIM], BF16, tag="bias_sb")
        nc.sync.dma_start(bias_sb[:], bias[:, :])
        c_sb = const.tile([P, BT, N_EXP], F32, tag="c_sb")
        nc.sync.dma_start(c_sb[:], c[:, :])
        xh_sb = const.tile([P, KT2, 2, BS], F8, tag="xh_sb")
        nc.sync.dma_start(xh_sb[:], xh8[:, :])

        ones8 = const.tile([P, 2, 16], F8, tag="ones8")
        nc.vector.memset(ones8[:], 1.0)
        junk8 = const.tile([P, 2, FD], F8, tag="junk8")
        nc.vector.memset(junk8[:], 1.0)
        rs_row = const.tile([1, BS], F32, tag="rs_row")
        rs_pb = const.tile([P, BT], F32, tag="rs_pb")
        r1_sb = const.tile([P, BT], F32, tag="r1_sb")
        sc_sb = const.tile([P, BT], F32, tag="sc_sb")

        acc = [
            accp.tile([P, NO, FD], F32, name=f"acc_{bt}", tag=f"acc_{bt}")
            for bt in range(BT)
        ]

        w_sb = {}

        def fetch_group(grp, chunked):
            for n in grp:
                w_sb[n] = wpool.tile(
                    [P, KT2, 2, OUT_DIM], F8, name=f"w_{n}", tag="w_sb"
                )
            if chunked:
                # interleave the experts' K-chunks: the kt loop needs both
                # experts' chunk kt before it can proceed
                for kt in range(KT2):
                    for n in grp:
                        nc.sync.dma_start(w_sb[n][:, kt], w8[n, :, :][:, kt])
            else:
                for n in grp:
                    nc.sync.dma_start(w_sb[n][:], w8[n, :, :])

        groups = [(2 * p, 2 * p + 1) for p in range(NPAIR - 1)] + [(14,), (15,)]
        fetch_group(groups[0], chunked=False)

        xl_sb = const.tile([P, KT2, 2, BS], F8, tag="xl_sb")
        nc.sync.dma_start(xl_sb[:], xl8[:, :])

        fetch_group(groups[1], chunked=False)

        nc.vector.tensor_reduce(
            sc_sb[:], c_sb[:], axis=mybir.AxisListType.X, op=mybir.AluOpType.add
        )

        # --- PE clock warm-up: keep the PE busy through the DMA window
        # so the seeds and first main matmuls run at full p-state.
        jk = psum.tile([P, NO, FD], F32, name="junk", tag="zp")
        for _ in range(10):
            nc.tensor.matmul(
                jk[0:1, 0, :],
                lhsT=ones8[:, :, 0:1],
                rhs=junk8[:],
                start=True,
                stop=True,
                perf_mode=DR,
            )

        # --- bias seed: pt = (c @ bias) per bt, K=16 bf16 matmuls -------
        # Runs in the startup DMA window while xh/w0 stream in. The
        # accumulator-init copies split across Vector and Scalar so the
        # seed-tile WAR (which gates the first main matmuls' PSUM slots)
        # clears right after the seeds.
        seed_pt = []
        for bt in range(BT):
            pt = psum.tile([P, NO, FD], F32, name=f"seed_{bt}", tag="zp")
            for ot in range(NO):
                nc.tensor.matmul(
                    pt[:, ot],
                    lhsT=cT_sb[:, bt * P : (bt + 1) * P],
                    rhs=bias_sb[:, ot * FD : (ot + 1) * FD],
                    start=True,
                    stop=True,
                )
            seed_pt.append(pt)
        for bt in range(BT):
            if bt % 2 == 0:
                nc.vector.tensor_copy(acc[bt][:], seed_pt[bt][:])
            else:
                nc.scalar.activation(acc[bt][:], seed_pt[bt][:], ACT.Copy)

        # --- main expert loop: pairs for 0-13, solo for 14/15 ----------
        out_ap = out[:, :]
        for gi, grp in enumerate(groups):
            for bt in range(BT):
                ne = len(grp)
                zps = [
                    psum.tile([P, NO, FD], F32, name=f"zp_{n}", tag="zp")
                    for n in grp
                ]
                if grp == (N_EXP - 1,):
                    # ot-major: close each ot's accumulation group early so
                    # the final combine/relu/store overlaps ot1's streaming
                    mm_order = [
                        (kt, 0, ot) for ot in range(NO) for kt in range(KT2)
                    ]
                elif gi == 0 and bt == 0:
                    # expert-major: expert 0 streams while expert 1's
                    # weight DMA is still in flight at startup
                    mm_order = [
                        (kt, ei, ot)
                        for ei in range(ne)
                        for kt in range(KT2)
                        for ot in range(NO)
                    ]
                else:
                    mm_order = [
                        (kt, ei, ot)
                        for kt in range(KT2)
                        for ei in range(ne)
                        for ot in range(NO)
                    ]
                for kt, ei, ot in mm_order:
                    nc.tensor.matmul(
                        zps[ei][:, ot],
                        lhsT=xh_sb[:, kt, :, bt * P : (bt + 1) * P],
                        rhs=w_sb[grp[ei]][:, kt, :, ot * FD : (ot + 1) * FD],
                        start=(kt == 0),
                        stop=(kt == KT2 - 1),
                        perf_mode=DR,
                    )
                for ei, n in enumerate(grp):
                    if n != N_EXP - 1:
                        nc.vector.scalar_tensor_tensor(
                            out=acc[bt][:],
                            in0=zps[ei][:],
                            scalar=c_sb[:, bt, n : n + 1],
                            in1=acc[bt][:],
                            op0=mybir.AluOpType.mult,
                            op1=mybir.AluOpType.add,
                        )
                    else:
                        # last expert: combine + relu(+rank-1 bias) + store
                        for ot in range(NO):
                            nc.vector.scalar_tensor_tensor(
                                out=acc[bt][:, ot],
                                in0=zps[ei][:, ot],
                                scalar=c_sb[:, bt, n : n + 1],
                                in1=acc[bt][:, ot],
                                op0=mybir.AluOpType.mult,
                                op1=mybir.AluOpType.add,
                            )
                            nc.scalar.activation(
                                acc[bt][:, ot],
                                acc[bt][:, ot],
                                ACT.Relu,
                                bias=r1_sb[:, bt : bt + 1],
                            )
                            nc.sync.dma_start(
                                out_ap[:, bt, ot * FD : (ot + 1) * FD],
                                acc[bt][:, ot],
                            )

            if gi == 0:
                # --- rowsum(x) = rowsum(xh) + rowsum(xl) ---------------
                # ones-stationary DoubleRow matmuls -> [1, 512] on
                # partition 0, transposed to [128, 4] via small DMAs.
                # Emitted after group 0 so it never gates the startup;
                # drained by Scalar and combined on GpSimd so the Vector
                # combine stream is untouched; r1 is consumed only by the
                # final ReLU bias.
                rs_pt = psum.tile([P, NO, FD], F32, name="rs", tag="zp")
                n_rs = 2 * KT2
                i_rs = 0
                for xt in (xh_sb, xl_sb):
                    for kt in range(KT2):
                        nc.tensor.matmul(
                            rs_pt[0:1, 0, :],
                            lhsT=ones8[:, :, 0:1],
                            rhs=xt[:, kt],
                            start=(i_rs == 0),
                            stop=(i_rs == n_rs - 1),
                            perf_mode=DR,
                        )
                        i_rs += 1
                nc.scalar.activation(rs_row[:], rs_pt[0:1, 0, :], ACT.Copy)
                for bt in range(BT):
                    nc.sync.dma_start(
                        rs_pb[:, bt : bt + 1], rs_row[0:1, bt * P : (bt + 1) * P]
                    )
                # r1 = 0.5 * rowsum * sum_c   (tiny op, [128, 4])
                nc.vector.scalar_tensor_tensor(
                    out=r1_sb[:],
                    in0=rs_pb[:],
                    scalar=0.5,
                    in1=sc_sb[:],
                    op0=mybir.AluOpType.mult,
                    op1=mybir.AluOpType.mult,
                )

            # prefetch two groups ahead: emitted after this group's
            # matmuls so the pool-slot WAR dependency sees the readers.
            if gi + 2 < len(groups):
                fetch_group(groups[gi + 2], chunked=False)

    nc.compile()
    return nc


_NC_CACHE = {}


def _get_nc():
    if "nc" not in _NC_CACHE:
        _NC_CACHE["nc"] = _build_kernel()
    return _NC_CACHE["nc"]


def _xt_layout(x8):
    # fp8 [BS, IN_DIM] -> lhsT [P, KT2, 2, BS] with k = kt2*256+slot*128+p
    xT = np.ascontiguousarray(x8.T)  # [IN_DIM, BS]
    return np.ascontiguousarray(xT.reshape(KT2, 2, P, BS).transpose(2, 0, 1, 3))


def prepare_inputs(x, comp_weight, weight, bias):
    x = np.ascontiguousarray(np.asarray(x, dtype=np.float32))
    comp_weight = np.ascontiguousarray(np.asarray(comp_weight, dtype=np.float32))
    weight = np.asarray(weight, dtype=np.float32)
    bias = np.ascontiguousarray(np.asarray(bias, dtype=np.float32))

    # w = 0.5 + v; ship v in fp8 laid out [n, p, kt2, slot, o]
    v8 = (weight - np.float32(0.5)).astype(E4NP)
    w8 = np.ascontiguousarray(
        v8.reshape(N_EXP, KT2, 2, P, OUT_DIM).transpose(0, 3, 1, 2, 4)
    )
    bias_bf = bias.astype(ml_dtypes.bfloat16)

    in_maps = []
    for r in range(N_CORES):
        sl = slice(r * BS, (r + 1) * BS)
        xs = x[sl]
        cs = comp_weight[sl]
        xh = xs.astype(E4NP)
        xl = (xs - xh.astype(np.float32)).astype(E4NP)
        in_maps.append(
            {
                "xh8": _xt_layout(xh),
                "xl8": _xt_layout(xl),
                "w8": w8,
                "c": np.ascontiguousarray(cs.reshape(BT, P, N_EXP).transpose(1, 0, 2)),
                "cT": np.ascontiguousarray(cs.T).astype(ml_dtypes.bfloat16),
                "bias": bias_bf,
            }
        )
    return in_maps


def _run(x, comp_weight, weight, bias, trace=False):
    in_maps = prepare_inputs(x, comp_weight, weight, bias)
    res = run_bass_kernel_spmd(
        _get_nc(), in_maps, core_ids=list(range(N_CORES)), trace=trace
    )
    out = np.concatenate(
        [
            res.results[r]["out"].transpose(1, 0, 2).reshape(BS, OUT_DIM)
            for r in range(N_CORES)
        ],
        axis=0,
    )
    return out, res


def kernel(x, comp_weight, weight, bias):
    out, _ = _run(x, comp_weight, weight, bias)
    return out


# revision 18
# speedup vs baseline: 1.1001x; 1.0019x over previous
"""Trainium2 Bass kernel for CompositionalFC (moe_routing).

Reference computation:
    z[n,b,o] = x[b,i] @ weight[n,i,o] + bias[n,o]
    out[b,o] = relu( sum_n comp_weight[b,n] * z[n,b,o] )

Strategy: data-parallel over batch across 8 NeuronCores (512 rows each,
weight/bias replicated), with the expert matmuls in fp8e4 DoubleRow mode
(2 contraction rows per PE pass = 2x bf16 matmul throughput, and half the
weight DMA traffic). Steady state measured at 216 ns per 512-col DoubleRow
matmul == the fp8 PE roofline (~157 TF/s effective per core).

Accuracy: fp8e4 has a 3-bit mantissa, too coarse for w ~ U[0,1) directly
(~3.4% rel err vs the 2e-2 gate). Mean-centering fixes it: w = 0.5 + v
with v ~ U[-.5,.5); quantize v to fp8 and add the exact rank-1 term
    0.5 * rowsum(x)[b] * (sum_n c[b,n]),
which also dominates the output magnitude. x ships as fp8 pair
x = xh + xl; the main pass uses xh only, while rowsum(x) is recovered as
rowsum(xh) + rowsum(xl) on device via ones-stationary DoubleRow matmuls
(single LdWeights, output [1, 512] transposed to [128, 4] by small
SBUF->SBUF DMAs). Measured end-to-end l2 rel err: 7.3e-3.

Per core: z_n accumulates in PSUM over 4 DoubleRow K-tiles of 256, then
one fused combine per expert on the Vector engine: acc = z*c[:,n] + acc.
The bias term (comp_weight @ bias) seeds the accumulators via K=16 bf16
matmuls, hidden in the startup DMA window behind PE-clock warm-up junk
matmuls; the rank-1 term is folded into the final ReLU's per-partition
bias on the Scalar engine.

Engine budget: Vector runs the full combines (~77% busy at steady state,
~2 us slack per expert group -- nothing big may sit in its queue);
Scalar (PSUM-capable) takes half the accumulator seeds, the rowsum
drain, and the final ReLUs. GpSimd is unused for tensor work (no PSUM
access on TRN2 and its tensor ops run at half Vector rate).
Expert groups: pairs for 0-13 sharing each stationary xh tile
across 2 experts x 2 PSUM banks (4 matmuls per LdWeights), then experts
14 and 15 solo so the final drain chains spread over the last two expert
windows instead of piling up behind the very last matmuls. Weight pair
DMAs interleave the two experts' K-chunks (the kt loop needs both
experts' chunk kt first); later groups prefetch two ahead through a
4-deep pool.
"""

import sys

for _p in ("/opt/trn_rl_repo",):
    if _p not in sys.path:
        sys.path.insert(0, _p)

from contextlib import ExitStack

import ml_dtypes
import numpy as np

import concourse.bass as bass
import concourse.mybir as mybir
import concourse.tile as tile
from concourse import bacc
from concourse.bass_utils import run_bass_kernel_spmd

N_CORES = 8
BATCH, IN_DIM, OUT_DIM, N_EXP = 4096, 1024, 1024, 16
BS = BATCH // N_CORES          # 512 batch rows per core
P = 128                        # partitions
BT = BS // P                   # 4 batch tiles per core
KT2 = IN_DIM // 256            # 4 DoubleRow contraction tiles (K=256 each)
FD = 512                       # matmul free dim / PSUM bank width (fp32)
NO = OUT_DIM // FD             # 2 output column tiles
NPAIR = N_EXP // 2

F32 = mybir.dt.float32
BF16 = mybir.dt.bfloat16
F8 = mybir.dt.float8e4
DR = mybir.MatmulPerfMode.DoubleRow
ACT = mybir.ActivationFunctionType

E4NP = ml_dtypes.float8_e4m3   # TRN fp8e4 == IEEE e4m3 (max 240)


def _build_kernel():
    nc = bacc.Bacc(
        "TRN2",
        target_bir_lowering=False,
        debug=False,
        num_devices=N_CORES,
    )
    # k = kt2*256 + slot*128 + p; b = bt*128 + p_out
    xh8 = nc.declare_dram_parameter("xh8", [P, KT2, 2, BS], F8, isOutput=False)
    xl8 = nc.declare_dram_parameter("xl8", [P, KT2, 2, BS], F8, isOutput=False)
    w8 = nc.declare_dram_parameter("w8", [N_EXP, P, KT2, 2, OUT_DIM], F8, isOutput=False)
    c = nc.declare_dram_parameter("c", [P, BT, N_EXP], F32, isOutput=False)
    cT = nc.declare_dram_parameter("cT", [N_EXP, BS], BF16, isOutput=False)
    bias = nc.declare_dram_parameter("bias", [N_EXP, OUT_DIM], BF16, isOutput=False)
    out = nc.declare_dram_parameter("out", [P, BT, OUT_DIM], F32, isOutput=True)

    with ExitStack() as ctx:
        tc = ctx.enter_context(tile.TileContext(nc))
        const = ctx.enter_context(tc.tile_pool(name="const", bufs=1))
        accp = ctx.enter_context(tc.tile_pool(name="accp", bufs=1))
        wpool = ctx.enter_context(tc.tile_pool(name="wpool", bufs=4))
        psum = ctx.enter_context(tc.tile_pool(name="psum", bufs=4, space="PSUM"))

        # --- startup DMAs, all on sync (GpSimd's DMA path has ~5us
        # completion latency), most-gating first ------------------------
        cT_sb = const.tile([N_EXP, BS], BF16, tag="cT_sb")
        nc.sync.dma_start(cT_sb[:], cT[:, :])
        bias_sb = const.tile([N_EXP, OUT_DIM], BF16, tag="bias_sb")
        nc.sync.dma_start(bias_sb[:], bias[:, :])
        c_sb = const.tile([P, BT, N_EXP], F32, tag="c_sb")
        nc.sync.dma_start(c_sb[:], c[:, :])
        xh_sb = const.tile([P, KT2, 2, BS], F8, tag="xh_sb")
        nc.sync.dma_start(xh_sb[:], xh8[:, :])

        ones8 = const.tile([P, 2, 16], F8, tag="ones8")
        nc.vector.memset(ones8[:], 1.0)
        junk8 = const.tile([P, 2, FD], F8, tag="junk8")
        nc.vector.memset(junk8[:], 1.0)
        rs_row = const.tile([1, BS], F32, tag="rs_row")
        rs_pb = const.tile([P, BT], F32, tag="rs_pb")
        r1_sb = const.tile([P, BT], F32, tag="r1_sb")
        sc_sb = const.tile([P, BT], F32, tag="sc_sb")

        acc = [
            accp.tile([P, NO, FD], F32, name=f"acc_{bt}", tag=f"acc_{bt}")
            for bt in range(BT)
        ]

        w_sb = {}

        def fetch_group(grp, halves=False):
            for n in grp:
                w_sb[n] = wpool.tile(
                    [P, KT2, 2, OUT_DIM], F8, name=f"w_{n}", tag="w_sb"
                )
            if halves:
                # K-halves so expert 0's first matmuls start while the
                # rest of the pair is still on the wire (startup only)
                for n in grp:
                    h = KT2 // 2
                    nc.sync.dma_start(w_sb[n][:, 0:h], w8[n, :, :][:, 0:h])
                    nc.sync.dma_start(w_sb[n][:, h:KT2], w8[n, :, :][:, h:KT2])
            else:
                for n in grp:
                    nc.sync.dma_start(w_sb[n][:], w8[n, :, :])

        groups = [(2 * p, 2 * p + 1) for p in range(NPAIR - 1)] + [(14,), (15,)]
        fetch_group(groups[0], halves=True)

        xl_sb = const.tile([P, KT2, 2, BS], F8, tag="xl_sb")
        nc.sync.dma_start(xl_sb[:], xl8[:, :])

        fetch_group(groups[1])

        nc.vector.tensor_reduce(
            sc_sb[:], c_sb[:], axis=mybir.AxisListType.X, op=mybir.AluOpType.add
        )

        # --- PE clock warm-up: keep the PE busy through the DMA window
        # so the seeds and first main matmuls run at full p-state.
        jk = psum.tile([P, NO, FD], F32, name="junk", tag="zp")
        for _ in range(10):
            nc.tensor.matmul(
                jk[0:1, 0, :],
                lhsT=ones8[:, :, 0:1],
                rhs=junk8[:],
                start=True,
                stop=True,
                perf_mode=DR,
            )

        # --- bias seed: pt = (c @ bias) per bt, K=16 bf16 matmuls -------
        # Runs in the startup DMA window while xh/w0 stream in. The
        # accumulator-init copies split across Vector and Scalar so the
        # seed-tile WAR (which gates the first main matmuls' PSUM slots)
        # clears right after the seeds.
        seed_pt = []
        for bt in range(BT):
            pt = psum.tile([P, NO, FD], F32, name=f"seed_{bt}", tag="zp")
            for ot in range(NO):
                nc.tensor.matmul(
                    pt[:, ot],
                    lhsT=cT_sb[:, bt * P : (bt + 1) * P],
                    rhs=bias_sb[:, ot * FD : (ot + 1) * FD],
                    start=True,
                    stop=True,
                )
            seed_pt.append(pt)
        for bt in range(BT):
            if bt % 2 == 0:
                nc.vector.tensor_copy(acc[bt][:], seed_pt[bt][:])
            else:
                nc.scalar.activation(acc[bt][:], seed_pt[bt][:], ACT.Copy)

        # --- main expert loop: pairs for 0-13, solo for 14/15 ----------
        out_ap = out[:, :]
        for gi, grp in enumerate(groups):
            for bt in range(BT):
                ne = len(grp)
                zps = [
                    psum.tile([P, NO, FD], F32, name=f"zp_{n}", tag="zp")
                    for n in grp
                ]
                if grp == (N_EXP - 1,):
                    # ot-major: close each ot's accumulation group early so
                    # the final combine/relu/store overlaps ot1's streaming
                    mm_order = [
                        (kt, 0, ot) for ot in range(NO) for kt in range(KT2)
                    ]
                elif gi == 0 and bt == 0:
                    # expert-major: expert 0 streams while expert 1's
                    # weight DMA is still in flight at startup (kt order
                    # matches the half-tile DMA arrival order)
                    mm_order = [
                        (kt, ei, ot)
                        for ei in range(ne)
                        for kt in range(KT2)
                        for ot in range(NO)
                    ]
                else:
                    mm_order = [
                        (kt, ei, ot)
                        for kt in range(KT2)
                        for ei in range(ne)
                        for ot in range(NO)
                    ]
                for kt, ei, ot in mm_order:
                    nc.tensor.matmul(
                        zps[ei][:, ot],
                        lhsT=xh_sb[:, kt, :, bt * P : (bt + 1) * P],
                        rhs=w_sb[grp[ei]][:, kt, :, ot * FD : (ot + 1) * FD],
                        start=(kt == 0),
                        stop=(kt == KT2 - 1),
                        perf_mode=DR,
                    )
                for ei, n in enumerate(grp):
                    if n != N_EXP - 1:
                        nc.vector.scalar_tensor_tensor(
                            out=acc[bt][:],
                            in0=zps[ei][:],
                            scalar=c_sb[:, bt, n : n + 1],
                            in1=acc[bt][:],
                            op0=mybir.AluOpType.mult,
                            op1=mybir.AluOpType.add,
                        )
                    else:
                        # last expert: combine + relu(+rank-1 bias) + store
                        for ot in range(NO):
                            nc.vector.scalar_tensor_tensor(
                                out=acc[bt][:, ot],
                                in0=zps[ei][:, ot],
                                scalar=c_sb[:, bt, n : n + 1],
                                in1=acc[bt][:, ot],
                                op0=mybir.AluOpType.mult,
                                op1=mybir.AluOpType.add,
                            )
                            nc.scalar.activation(
                                acc[bt][:, ot],
                                acc[bt][:, ot],
                                ACT.Relu,
                                bias=r1_sb[:, bt : bt + 1],
                            )
                            nc.sync.dma_start(
                                out_ap[:, bt, ot * FD : (ot + 1) * FD],
                                acc[bt][:, ot],
                            )

            if gi == 0:
                # --- rowsum(x) = rowsum(xh) + rowsum(xl) ---------------
                # ones-stationary DoubleRow matmuls -> [1, 512] on
                # partition 0, transposed to [128, 4] via small DMAs.
                # Emitted after group 0 so it never gates the startup;
                # drained by Scalar and combined on GpSimd so the Vector
                # combine stream is untouched; r1 is consumed only by the
                # final ReLU bias.
                rs_pt = psum.tile([P, NO, FD], F32, name="rs", tag="zp")
                n_rs = 2 * KT2
                i_rs = 0
                for xt in (xh_sb, xl_sb):
                    for kt in range(KT2):
                        nc.tensor.matmul(
                            rs_pt[0:1, 0, :],
                            lhsT=ones8[:, :, 0:1],
                            rhs=xt[:, kt],
                            start=(i_rs == 0),
                            stop=(i_rs == n_rs - 1),
                            perf_mode=DR,
                        )
                        i_rs += 1
                nc.scalar.activation(rs_row[:], rs_pt[0:1, 0, :], ACT.Copy)
                for bt in range(BT):
                    nc.sync.dma_start(
                        rs_pb[:, bt : bt + 1], rs_row[0:1, bt * P : (bt + 1) * P]
                    )
                # r1 = 0.5 * rowsum * sum_c   (tiny op, [128, 4])
                nc.vector.scalar_tensor_tensor(
                    out=r1_sb[:],
                    in0=rs_pb[:],
                    scalar=0.5,
                    in1=sc_sb[:],
                    op0=mybir.AluOpType.mult,
                    op1=mybir.AluOpType.mult,
                )

            # prefetch two groups ahead: emitted after this group's
            # matmuls so the pool-slot WAR dependency sees the readers.
            if gi + 2 < len(groups):
                fetch_group(groups[gi + 2])

    nc.compile()
    return nc


_NC_CACHE = {}


def _get_nc():
    if "nc" not in _NC_CACHE:
        _NC_CACHE["nc"] = _build_kernel()
    return _NC_CACHE["nc"]


def _xt_layout(x8):
    # fp8 [BS, IN_DIM] -> lhsT [P, KT2, 2, BS] with k = kt2*256+slot*128+p
    xT = np.ascontiguousarray(x8.T)  # [IN_DIM, BS]
    return np.ascontiguousarray(xT.reshape(KT2, 2, P, BS).transpose(2, 0, 1, 3))


def prepare_inputs(x, comp_weight, weight, bias):
    x = np.ascontiguousarray(np.asarray(x, dtype=np.float32))
    comp_weight = np.ascontiguousarray(np.asarray(comp_weight, dtype=np.float32))
    weight = np.asarray(weight, dtype=np.float32)
    bias = np.ascontiguousarray(np.asarray(bias, dtype=np.float32))

    # w = 0.5 + v; ship v in fp8 laid out [n, p, kt2, slot, o]
    v8 = (weight - np.float32(0.5)).astype(E4NP)
    w8 = np.ascontiguousarray(
        v8.reshape(N_EXP, KT2, 2, P, OUT_DIM).transpose(0, 3, 1, 2, 4)
    )
    bias_bf = bias.astype(ml_dtypes.bfloat16)

    in_maps = []
    for r in range(N_CORES):
        sl = slice(r * BS, (r + 1) * BS)
        xs = x[sl]
        cs = comp_weight[sl]
        xh = xs.astype(E4NP)
        xl = (xs - xh.astype(np.float32)).astype(E4NP)
        in_maps.append(
            {
                "xh8": _xt_layout(xh),
                "xl8": _xt_layout(xl),
                "w8": w8,
                "c": np.ascontiguousarray(cs.reshape(BT, P, N_EXP).transpose(1, 0, 2)),
                "cT": np.ascontiguousarray(cs.T).astype(ml_dtypes.bfloat16),
                "bias": bias_bf,
            }
        )
    return in_maps


def _run(x, comp_weight, weight, bias, trace=False):
    in_maps = prepare_inputs(x, comp_weight, weight, bias)
    res = run_bass_kernel_spmd(
        _get_nc(), in_maps, core_ids=list(range(N_CORES)), trace=trace
    )
    out = np.concatenate(
        [
            res.results[r]["out"].transpose(1, 0, 2).reshape(BS, OUT_DIM)
            for r in range(N_CORES)
        ],
        axis=0,
    )
    return out, res


def kernel(x, comp_weight, weight, bias):
    out, _ = _run(x, comp_weight, weight, bias)
    return out


# revision 19
# speedup vs baseline: 1.1095x; 1.0086x over previous
"""Trainium2 Bass kernel for CompositionalFC (moe_routing).

Reference computation:
    z[n,b,o] = x[b,i] @ weight[n,i,o] + bias[n,o]
    out[b,o] = relu( sum_n comp_weight[b,n] * z[n,b,o] )

Strategy: data-parallel over batch across 8 NeuronCores (512 rows each,
weight/bias replicated), with the expert matmuls in fp8e4 DoubleRow mode
(2 contraction rows per PE pass = 2x bf16 matmul throughput, and half the
weight DMA traffic). Steady state measured at 216 ns per 512-col DoubleRow
matmul == the fp8 PE roofline (~157 TF/s effective per core).

Accuracy: fp8e4 has a 3-bit mantissa, too coarse for w ~ U[0,1) directly
(~3.4% rel err vs the 2e-2 gate). Mean-centering fixes it: w = 0.5 + v
with v ~ U[-.5,.5); quantize v to fp8 and add the exact rank-1 term
    0.5 * rowsum(x)[b] * (sum_n c[b,n]),
which also dominates the output magnitude. x ships as fp8 pair
x = xh + xl; the main pass uses xh only, while rowsum(x) is recovered as
rowsum(xh) + rowsum(xl) on device via ones-stationary DoubleRow matmuls
(single LdWeights, output [1, 512] transposed to [128, 4] by small
SBUF->SBUF DMAs). Measured end-to-end l2 rel err: 7.3e-3.

Per core: z_n accumulates in PSUM over 4 DoubleRow K-tiles of 256, then
one fused combine per expert on the Vector engine: acc = z*c[:,n] + acc.
The bias term (comp_weight @ bias) seeds the accumulators via K=16 bf16
matmuls, hidden in the startup DMA window behind PE-clock warm-up junk
matmuls; the rank-1 term is folded into the final ReLU's per-partition
bias on the Scalar engine.

Engine budget: Vector runs the full combines (~77% busy at steady state,
~2 us slack per expert group -- nothing big may sit in its queue);
Scalar (PSUM-capable) takes half the accumulator seeds, the rowsum
drain, and the final ReLUs. GpSimd is unused for tensor work (no PSUM
access on TRN2 and its tensor ops run at half Vector rate).
Expert groups: pairs for 0-13 sharing each stationary xh tile
across 2 experts x 2 PSUM banks (4 matmuls per LdWeights), then experts
14 and 15 solo so the final drain chains spread over the last two expert
windows instead of piling up behind the very last matmuls. Weight pair
DMAs interleave the two experts' K-chunks (the kt loop needs both
experts' chunk kt first); later groups prefetch two ahead through a
4-deep pool.
"""

import sys

for _p in ("/opt/trn_rl_repo",):
    if _p not in sys.path:
        sys.path.insert(0, _p)

from contextlib import ExitStack

import ml_dtypes
import numpy as np

import concourse.bass as bass
import concourse.mybir as mybir
import concourse.tile as tile
from concourse import bacc
from concourse.bass_utils import run_bass_kernel_spmd

N_CORES = 8
BATCH, IN_DIM, OUT_DIM, N_EXP = 4096, 1024, 1024, 16
BS = BATCH // N_CORES          # 512 batch rows per core
P = 128                        # partitions
BT = BS // P                   # 4 batch tiles per core
KT2 = IN_DIM // 256            # 4 DoubleRow contraction tiles (K=256 each)
FD = 512                       # matmul free dim / PSUM bank width (fp32)
NO = OUT_DIM // FD             # 2 output column tiles
NPAIR = N_EXP // 2

F32 = mybir.dt.float32
BF16 = mybir.dt.bfloat16
F8 = mybir.dt.float8e4
DR = mybir.MatmulPerfMode.DoubleRow
ACT = mybir.ActivationFunctionType

E4NP = ml_dtypes.float8_e4m3   # TRN fp8e4 == IEEE e4m3 (max 240)


def _build_kernel():
    nc = bacc.Bacc(
        "TRN2",
        target_bir_lowering=False,
        debug=False,
        num_devices=N_CORES,
    )
    # k = kt2*256 + slot*128 + p; b = bt*128 + p_out
    xh8 = nc.declare_dram_parameter("xh8", [P, KT2, 2, BS], F8, isOutput=False)
    xl8 = nc.declare_dram_parameter("xl8", [P, KT2, 2, BS], F8, isOutput=False)
    w8 = nc.declare_dram_parameter("w8", [N_EXP, P, KT2, 2, OUT_DIM], F8, isOutput=False)
    c = nc.declare_dram_parameter("c", [P, BT, N_EXP], F32, isOutput=False)
    cT = nc.declare_dram_parameter("cT", [N_EXP, BS], BF16, isOutput=False)
    bias = nc.declare_dram_parameter("bias", [N_EXP, OUT_DIM], BF16, isOutput=False)
    out = nc.declare_dram_parameter("out", [P, BT, OUT_DIM], BF16, isOutput=True)

    with ExitStack() as ctx:
        tc = ctx.enter_context(tile.TileContext(nc))
        const = ctx.enter_context(tc.tile_pool(name="const", bufs=1))
        accp = ctx.enter_context(tc.tile_pool(name="accp", bufs=1))
        wpool = ctx.enter_context(tc.tile_pool(name="wpool", bufs=4))
        psum = ctx.enter_context(tc.tile_pool(name="psum", bufs=4, space="PSUM"))

        # --- startup DMAs, all on sync (GpSimd's DMA path has ~5us
        # completion latency), most-gating first ------------------------
        cT_sb = const.tile([N_EXP, BS], BF16, tag="cT_sb")
        nc.sync.dma_start(cT_sb[:], cT[:, :])
        bias_sb = const.tile([N_EXP, OUT_DIM], BF16, tag="bias_sb")
        nc.sync.dma_start(bias_sb[:], bias[:, :])
        c_sb = const.tile([P, BT, N_EXP], F32, tag="c_sb")
        nc.sync.dma_start(c_sb[:], c[:, :])
        xh_sb = const.tile([P, KT2, 2, BS], F8, tag="xh_sb")
        nc.sync.dma_start(xh_sb[:], xh8[:, :])

        ones8 = const.tile([P, 2, 16], F8, tag="ones8")
        nc.vector.memset(ones8[:], 1.0)
        junk8 = const.tile([P, 2, FD], F8, tag="junk8")
        nc.vector.memset(junk8[:], 1.0)
        rs_row = const.tile([1, BS], F32, tag="rs_row")
        rs_pb = const.tile([P, BT], F32, tag="rs_pb")
        r1_sb = const.tile([P, BT], F32, tag="r1_sb")
        sc_sb = const.tile([P, BT], F32, tag="sc_sb")

        acc = [
            accp.tile([P, NO, FD], F32, name=f"acc_{bt}", tag=f"acc_{bt}")
            for bt in range(BT)
        ]
        ob_sb = [
            accp.tile([P, NO, FD], BF16, name=f"ob_{bt}", tag=f"ob_{bt}")
            for bt in range(BT)
        ]

        w_sb = {}

        def fetch_group(grp, halves=False):
            for n in grp:
                w_sb[n] = wpool.tile(
                    [P, KT2, 2, OUT_DIM], F8, name=f"w_{n}", tag="w_sb"
                )
            if halves:
                # K-halves so expert 0's first matmuls start while the
                # rest of the pair is still on the wire (startup only)
                for n in grp:
                    h = KT2 // 2
                    nc.sync.dma_start(w_sb[n][:, 0:h], w8[n, :, :][:, 0:h])
                    nc.sync.dma_start(w_sb[n][:, h:KT2], w8[n, :, :][:, h:KT2])
            else:
                for n in grp:
                    nc.sync.dma_start(w_sb[n][:], w8[n, :, :])

        groups = [(2 * p, 2 * p + 1) for p in range(NPAIR - 1)] + [(14,), (15,)]
        fetch_group(groups[0], halves=True)

        xl_sb = const.tile([P, KT2, 2, BS], F8, tag="xl_sb")
        nc.sync.dma_start(xl_sb[:], xl8[:, :])

        fetch_group(groups[1])

        nc.vector.tensor_reduce(
            sc_sb[:], c_sb[:], axis=mybir.AxisListType.X, op=mybir.AluOpType.add
        )

        # --- PE clock warm-up: keep the PE busy through the DMA window
        # so the seeds and first main matmuls run at full p-state.
        jk = psum.tile([P, NO, FD], F32, name="junk", tag="zp")
        for _ in range(10):
            nc.tensor.matmul(
                jk[0:1, 0, :],
                lhsT=ones8[:, :, 0:1],
                rhs=junk8[:],
                start=True,
                stop=True,
                perf_mode=DR,
            )

        # --- bias seed: pt = (c @ bias) per bt, K=16 bf16 matmuls -------
        # Runs in the startup DMA window while xh/w0 stream in. The
        # accumulator-init copies split across Vector and Scalar so the
        # seed-tile WAR (which gates the first main matmuls' PSUM slots)
        # clears right after the seeds.
        seed_pt = []
        for bt in range(BT):
            pt = psum.tile([P, NO, FD], F32, name=f"seed_{bt}", tag="zp")
            for ot in range(NO):
                nc.tensor.matmul(
                    pt[:, ot],
                    lhsT=cT_sb[:, bt * P : (bt + 1) * P],
                    rhs=bias_sb[:, ot * FD : (ot + 1) * FD],
                    start=True,
                    stop=True,
                )
            seed_pt.append(pt)
        for bt in range(BT):
            if bt % 2 == 0:
                nc.vector.tensor_copy(acc[bt][:], seed_pt[bt][:])
            else:
                nc.scalar.activation(acc[bt][:], seed_pt[bt][:], ACT.Copy)
        # keep the PE p-state hot through the remaining weight-DMA wait;
        # writes recycle seed 0's region (WAR-ordered after its drain)
        for _ in range(6):
            nc.tensor.matmul(
                seed_pt[0][0:1, 0, :],
                lhsT=ones8[:, :, 0:1],
                rhs=junk8[:],
                start=True,
                stop=True,
                perf_mode=DR,
            )

        # --- main expert loop: pairs for 0-13, solo for 14/15 ----------
        out_ap = out[:, :]
        for gi, grp in enumerate(groups):
            for bt in range(BT):
                ne = len(grp)
                zps = [
                    psum.tile([P, NO, FD], F32, name=f"zp_{n}", tag="zp")
                    for n in grp
                ]
                if grp == (N_EXP - 1,):
                    # ot-major: close each ot's accumulation group early so
                    # the final combine/relu/store overlaps ot1's streaming
                    mm_order = [
                        (kt, 0, ot) for ot in range(NO) for kt in range(KT2)
                    ]
                elif gi == 0 and bt == 0:
                    # expert-major: expert 0 streams while expert 1's
                    # weight DMA is still in flight at startup (kt order
                    # matches the half-tile DMA arrival order)
                    mm_order = [
                        (kt, ei, ot)
                        for ei in range(ne)
                        for kt in range(KT2)
                        for ot in range(NO)
                    ]
                else:
                    mm_order = [
                        (kt, ei, ot)
                        for kt in range(KT2)
                        for ei in range(ne)
                        for ot in range(NO)
                    ]
                for kt, ei, ot in mm_order:
                    nc.tensor.matmul(
                        zps[ei][:, ot],
                        lhsT=xh_sb[:, kt, :, bt * P : (bt + 1) * P],
                        rhs=w_sb[grp[ei]][:, kt, :, ot * FD : (ot + 1) * FD],
                        start=(kt == 0),
                        stop=(kt == KT2 - 1),
                        perf_mode=DR,
                    )
                for ei, n in enumerate(grp):
                    if n != N_EXP - 1:
                        nc.vector.scalar_tensor_tensor(
                            out=acc[bt][:],
                            in0=zps[ei][:],
                            scalar=c_sb[:, bt, n : n + 1],
                            in1=acc[bt][:],
                            op0=mybir.AluOpType.mult,
                            op1=mybir.AluOpType.add,
                        )
                    else:
                        # last expert: combine + relu(+rank-1 bias) + store
                        for ot in range(NO):
                            nc.vector.scalar_tensor_tensor(
                                out=acc[bt][:, ot],
                                in0=zps[ei][:, ot],
                                scalar=c_sb[:, bt, n : n + 1],
                                in1=acc[bt][:, ot],
                                op0=mybir.AluOpType.mult,
                                op1=mybir.AluOpType.add,
                            )
                            nc.scalar.activation(
                                ob_sb[bt][:, ot],
                                acc[bt][:, ot],
                                ACT.Relu,
                                bias=r1_sb[:, bt : bt + 1],
                            )
                            nc.sync.dma_start(
                                out_ap[:, bt, ot * FD : (ot + 1) * FD],
                                ob_sb[bt][:, ot],
                            )

            if gi == 0:
                # --- rowsum(x) = rowsum(xh) + rowsum(xl) ---------------
                # ones-stationary DoubleRow matmuls -> [1, 512] on
                # partition 0, transposed to [128, 4] via small DMAs.
                # Emitted after group 0 so it never gates the startup;
                # drained by Scalar and combined on GpSimd so the Vector
                # combine stream is untouched; r1 is consumed only by the
                # final ReLU bias.
                rs_pt = psum.tile([P, NO, FD], F32, name="rs", tag="zp")
                n_rs = 2 * KT2
                i_rs = 0
                for xt in (xh_sb, xl_sb):
                    for kt in range(KT2):
                        nc.tensor.matmul(
                            rs_pt[0:1, 0, :],
                            lhsT=ones8[:, :, 0:1],
                            rhs=xt[:, kt],
                            start=(i_rs == 0),
                            stop=(i_rs == n_rs - 1),
                            perf_mode=DR,
                        )
                        i_rs += 1
                nc.scalar.activation(rs_row[:], rs_pt[0:1, 0, :], ACT.Copy)
                for bt in range(BT):
                    nc.sync.dma_start(
                        rs_pb[:, bt : bt + 1], rs_row[0:1, bt * P : (bt + 1) * P]
                    )
                # r1 = 0.5 * rowsum * sum_c   (tiny op, [128, 4])
                nc.vector.scalar_tensor_tensor(
                    out=r1_sb[:],
                    in0=rs_pb[:],
                    scalar=0.5,
                    in1=sc_sb[:],
                    op0=mybir.AluOpType.mult,
                    op1=mybir.AluOpType.mult,
                )

            # prefetch two groups ahead: emitted after this group's
            # matmuls so the pool-slot WAR dependency sees the readers.
            if gi + 2 < len(groups):
                fetch_group(groups[gi + 2])

    nc.compile()
    return nc


_NC_CACHE = {}


def _get_nc():
    if "nc" not in _NC_CACHE:
        _NC_CACHE["nc"] = _build_kernel()
    return _NC_CACHE["nc"]


def _xt_layout(x8):
    # fp8 [BS, IN_DIM] -> lhsT [P, KT2, 2, BS] with k = kt2*256+slot*128+p
    xT = np.ascontiguousarray(x8.T)  # [IN_DIM, BS]
    return np.ascontiguousarray(xT.reshape(KT2, 2, P, BS).transpose(2, 0, 1, 3))


def prepare_inputs(x, comp_weight, weight, bias):
    x = np.ascontiguousarray(np.asarray(x, dtype=np.float32))
    comp_weight = np.ascontiguousarray(np.asarray(comp_weight, dtype=np.float32))
    weight = np.asarray(weight, dtype=np.float32)
    bias = np.ascontiguousarray(np.asarray(bias, dtype=np.float32))

    # w = 0.5 + v; ship v in fp8 laid out [n, p, kt2, slot, o]
    v8 = (weight - np.float32(0.5)).astype(E4NP)
    w8 = np.ascontiguousarray(
        v8.reshape(N_EXP, KT2, 2, P, OUT_DIM).transpose(0, 3, 1, 2, 4)
    )
    bias_bf = bias.astype(ml_dtypes.bfloat16)

    in_maps = []
    for r in range(N_CORES):
        sl = slice(r * BS, (r + 1) * BS)
        xs = x[sl]
        cs = comp_weight[sl]
        xh = xs.astype(E4NP)
        xl = (xs - xh.astype(np.float32)).astype(E4NP)
        in_maps.append(
            {
                "xh8": _xt_layout(xh),
                "xl8": _xt_layout(xl),
                "w8": w8,
                "c": np.ascontiguousarray(cs.reshape(BT, P, N_EXP).transpose(1, 0, 2)),
                "cT": np.ascontiguousarray(cs.T).astype(ml_dtypes.bfloat16),
                "bias": bias_bf,
            }
        )
    return in_maps


def _run(x, comp_weight, weight, bias, trace=False):
    in_maps = prepare_inputs(x, comp_weight, weight, bias)
    res = run_bass_kernel_spmd(
        _get_nc(), in_maps, core_ids=list(range(N_CORES)), trace=trace
    )
    out = np.concatenate(
        [
            res.results[r]["out"]
            .astype(np.float32)
            .transpose(1, 0, 2)
            .reshape(BS, OUT_DIM)
            for r in range(N_CORES)
        ],
        axis=0,
    )
    return out, res


def kernel(x, comp_weight, weight, bias):
    out, _ = _run(x, comp_weight, weight, bias)
    return out
